# revision 16
# baseline (speedup 1.0000x reference)
"""Trainium2 Bass kernel for nn_Attention_13778255085887 — stage 2.

Dense multi-head attention block (EfficientViT-style):
  qkv 1x1 conv -> per-head softmax(q^T k * scale) -> v @ attn^T
  + depthwise conv(k=3) positional encoding on v -> proj 1x1 conv.

Shapes: B=8, dim=256, L=1024, heads=8, key_dim=16, head_dim=32.

Data-parallel over B across the 8 NeuronCores (zero collectives).

Stage-2 design vs the 93us baseline:
  - exp split across ScalarE (ACTIVATE) and VectorE (custom fused DVE op
    EXP4S_ANT: monic-cubic^4 in one 1-elem/cycle pass) so neither engine
    owns the full 8.4M-element softmax.
  - the softmax denominator rides the AV matmul as a 33rd ones-column of
    the v^T stationary (d lands on partitions 32/96 of the av psum tile),
    eliminating the ones^T@E "d quad" that used to re-stream every E tile
    through the PE (-65536 PE cycles).
  - AV quads run one step LAGGED (step i emits AV for step i-1's E) so the
    PE never waits on exp and keeps its p-state high.
  - finish: one full-tile reciprocal_approx_fast + gpsimd partition
    broadcasts of the 1/d rows + per-head multiplies (gpsimd off the
    critical path, VectorE for the last group's tail).
PSUM budget: 2 S slots (4 banks) + double-buffered (128,1024) av (4).
"""

import os

import ml_dtypes
import numpy as np

import concourse.bass as bass
import concourse.mybir as mybir
import concourse.tile as tile
from concourse import bacc
from concourse.bass_utils import run_bass_kernel_spmd

BF16 = mybir.dt.bfloat16
F32 = mybir.dt.float32
AF = mybir.ActivationFunctionType
ALU = mybir.AluOpType

NH, KD, HD = 8, 16, 32
DIM, L, B = 256, 1024, 8
SCALE = KD ** -0.5  # 0.25

# ---- custom fused DVE op: E = exp(S/4) in one VectorE pass ----------------
# exp(S/4) = (k*(S^3 + A S^2 + B S + C))^4 with the monic cubic approximating
# exp(S/16)/k on S in [-7, 7] (actual |S| <= 6); rel err ~8e-4 + bf16 round.
# 8 ALU stages: add,mul,add,mul,add,sq,sq,mul(C3); C3=k^4 spills via in1.
EXP_A = 49.225494348115454
EXP_B = 1551.4792245875115
EXP_C = 24808.966983048205
EXP_K4 = 2.637892201733194e-18


def _register_exp_op():
    import concourse.dve_ops as dve_ops
    from concourse.dve_ops import DveOp
    from concourse.dve_spec import (
        C0, C1, C2, C3, Spec, Src0, _has_src1, _spill_c3_to_src1, lower, sq,
    )
    from concourse.dve_uop import DveOpSpec

    name = "EXP4S_ANT"
    if name in dve_ops._SUB_OPCODE_FOR_NAME:
        return next(op for op in dve_ops.OPS if op.name == name)
    row = dve_ops._CUSTOM_DVE_ROW_BASE + len(dve_ops.OPS)
    assert row < 0x20
    dve_ops._SUB_OPCODE_FOR_NAME[name] = row
    body = _spill_c3_to_src1(
        sq(sq(((Src0 + C0) * Src0 + C1) * Src0 + C2)) * C3
    )
    spec = Spec(
        body=body,
        reference=lambda in0, in1, s0, s1, imm2:
            (((((in0 + s0) * in0 + s1) * in0 + imm2) ** 2) ** 2) * in1,
    )
    shas = {
        ver: DveOpSpec(name=name, opcode=row, uops=lower(spec, ver=ver),
                       rd1_en=_has_src1(spec)).sha(ver)
        for ver in ("v3", "v4")
    }
    op = DveOp(name, spec, subdim=False, uops_sha=shas)
    dve_ops.OPS.append(op)
    dve_ops.CUSTOM_DVE_SPECS[name] = op.spec
    return op


EXP_OP = _register_exp_op()

# steps whose p=1 exp tile runs on VectorE (the rest stay on ScalarE, which
# keeps both tiles on the startup/finish/transition steps)
DVE_STEPS = frozenset(
    [3, 4, 5, 6, 9, 10, 11, 12, 13, 14, 17, 18, 19, 20, 21, 22,
     25, 26, 27, 28, 29]
)


def _install_ntff_shim():
    """Optionally register the axon NTFF profiling hook (for trace=True)."""
    import sys
    import types

    name = "antenv.axon_hooks"
    if name in sys.modules:
        return
    try:
        import antenv
        from trn_agent_boot.trn_boot import _ntff_profile_via_ctypes
    except ImportError:
        return
    hooks = types.ModuleType(name)
    hooks._the_hook = None
    hooks.set_axon_ntff_profile_hook = lambda h: setattr(hooks, "_the_hook", h)
    hooks.get_axon_ntff_profile_hook = lambda: hooks._the_hook
    sys.modules[name] = hooks
    antenv.axon_hooks = hooks
    so = "/opt/axon/libaxon_pjrt.so"
    if os.path.exists(so):
        hook = _ntff_profile_via_ctypes(so)
        if hook is not None:
            hooks.set_axon_ntff_profile_hook(hook)


def build_kernel() -> bass.Bass:
    nc = bacc.Bacc("TRN2", target_bir_lowering=False, debug=False, num_devices=8)

    # ---- DRAM I/O (per-core shard; weights replicated) ----
    x_d = nc.dram_tensor("x", (128, 2, 1024), BF16, kind="ExternalInput")
    wk_d = nc.dram_tensor("wk", (128, 2, 256), BF16, kind="ExternalInput")
    wq_d = nc.dram_tensor("wq", (128, 2, 256), BF16, kind="ExternalInput")
    wv_d = nc.dram_tensor("wv", (128, 2, 256), BF16, kind="ExternalInput")
    # wv64 columns per head: [v(32) | zeros(32)]; zero cols + bias 1.0 make
    # v^T cols 32..63 exact ones columns (denominator ride-along: every one of
    # them accumulates d, so the whole av tile stays finite for the recip).
    wv64_d = nc.dram_tensor("wv64", (128, 2, 512), BF16, kind="ExternalInput")
    wpt_d = nc.dram_tensor("wpt", (128, 2, 256), BF16, kind="ExternalInput")
    # [bk(2) | bq(2) | bv(2) | wpe(6) | bpe(2) | bproj(2) | k4(1)]
    pf_d = nc.dram_tensor("pf", (128, 17), F32, kind="ExternalInput")
    bvrow_d = nc.dram_tensor("bvrow", (1, 512), F32, kind="ExternalInput")
    out_d = nc.dram_tensor("out", (128, 2, 1024), BF16, kind="ExternalOutput")

    with tile.TileContext(nc) as tc:
        with (
            tc.tile_pool(name="const", bufs=1) as cpool,
            tc.tile_pool(name="work", bufs=3) as wpool,
            tc.tile_pool(name="epool", bufs=10) as epool,
            tc.tile_pool(name="ps_s", bufs=2, space="PSUM") as ps_s,
            tc.tile_pool(name="ps_av", bufs=2, space="PSUM") as ps_av,
        ):
            # phase-1/3 matmuls borrow the S pool's (128, 1024) slots.
            _misc_n = [0]

            def misc_ps(width=512):
                _misc_n[0] += 1
                return ps_s.tile(
                    [128, 1024], F32, tag="S", name=f"misc{_misc_n[0]}"
                )[:, :width]

            # ---- load constants / activations ----
            x_sb = cpool.tile([128, 2, 1024], BF16, tag="x")
            wk = cpool.tile([128, 2, 256], BF16, tag="wk")
            wq = cpool.tile([128, 2, 256], BF16, tag="wq")
            wv = cpool.tile([128, 2, 256], BF16, tag="wv")
            wv64 = cpool.tile([128, 2, 512], BF16, tag="wv64")
            wpt = cpool.tile([128, 2, 256], BF16, tag="wpt")
            pf = cpool.tile([128, 17], F32, tag="pf")
            bvrow = cpool.tile([1, 512], F32, tag="bvrow")
            bk, bq, bv = pf[:, 0:2], pf[:, 2:4], pf[:, 4:6]
            wpe = pf[:, 6:12].rearrange("p (t k) -> p t k", t=2)
            bpe, bproj = pf[:, 12:14], pf[:, 14:16]
            k4c = pf[:, 16:17]

            # critical-path loads on the sync queue; bulk on the Pool queue
            nc.sync.dma_start(wk[:, :, :128], wk_d.ap()[:, :, :128])
            nc.sync.dma_start(wq[:, :, :128], wq_d.ap()[:, :, :128])
            nc.sync.dma_start(pf[:], pf_d.ap())
            nc.sync.dma_start(x_sb[:, :, :512], x_d.ap()[:, :, :512])
            nc.gpsimd.dma_start(wv64[:], wv64_d.ap())
            nc.sync.dma_start(wk[:, :, 128:], wk_d.ap()[:, :, 128:])
            nc.sync.dma_start(wq[:, :, 128:], wq_d.ap()[:, :, 128:])
            nc.gpsimd.dma_start(x_sb[:, :, 512:], x_d.ap()[:, :, 512:])
            nc.gpsimd.dma_start(wv[:], wv_d.ap())
            nc.gpsimd.dma_start(wpt[:], wpt_d.ap())
            nc.gpsimd.dma_start(bvrow[:], bvrow_d.ap())

            # persistent intermediates
            tk = cpool.tile([128, 2, 1024], BF16, tag="tk")        # packed k
            tq = cpool.tile([128, 2, 1024], BF16, tag="tq")        # packed q
            vnat = cpool.tile([128, 2, 1024], BF16, tag="vnat")    # v, natural
            vT = cpool.tile([128, 8, 512], BF16, tag="vT")         # v^T + ones
            bvb = cpool.tile([128, 512], F32, tag="bvb")           # b_v row-bcast
            peacc = cpool.tile([128, 2, 1024], BF16, tag="peacc")  # pe conv terms
            ybf = cpool.tile([128, 2, 1024], BF16, tag="ybf")      # y = av*R + pe
            zout = cpool.tile([128, 2, 1024], BF16, tag="zout")

            nc.gpsimd.partition_broadcast(bvb[:], bvrow[:], channels=128)

            # ---- phase-1 building blocks (emitted piecemeal) ----
            kq_ready = set()
            vt_ready = set()
            vn_ready = set()

            def emit_kq(t, n, w_sb, b_sb, dst):
                kq_ready.add((id(dst), t, n))
                ps = misc_ps()
                for kc in range(2):
                    nc.tensor.matmul(
                        ps[:], w_sb[:, kc, t * 128:(t + 1) * 128],
                        x_sb[:, kc, n * 512:(n + 1) * 512],
                        start=(kc == 0), stop=(kc == 1),
                    )
                nc.vector.tensor_scalar(
                    dst[:, t, n * 512:(n + 1) * 512], ps[:],
                    b_sb[:, t:t + 1], None, ALU.add,
                )

            def emit_kq_merged(t, w_sb, b_sb, dst):
                kq_ready.add((id(dst), t, 0))
                kq_ready.add((id(dst), t, 1))
                ps = misc_ps(1024)
                for n in range(2):
                    for kc in range(2):
                        nc.tensor.matmul(
                            ps[:, n * 512:(n + 1) * 512],
                            w_sb[:, kc, t * 128:(t + 1) * 128],
                            x_sb[:, kc, n * 512:(n + 1) * 512],
                            start=(kc == 0), stop=(kc == 1),
                            skip_group_check=True,
                        )
                nc.vector.tensor_scalar(
                    dst[:, t, :], ps[:], b_sb[:, t:t + 1], None, ALU.add,
                )

            def emit_kq01_pair():
                # n=1 halves of t=0 k and q, one borrow
                kq_ready.add((id(tk), 0, 1))
                kq_ready.add((id(tq), 0, 1))
                ps = misc_ps(1024)
                for h, w_sb in enumerate([wk, wq]):
                    for kc in range(2):
                        nc.tensor.matmul(
                            ps[:, h * 512:(h + 1) * 512],
                            w_sb[:, kc, :128],
                            x_sb[:, kc, 512:],
                            start=(kc == 0), stop=(kc == 1),
                            skip_group_check=True,
                        )
                nc.vector.tensor_scalar(
                    tk[:, 0, 512:], ps[:, :512], bk[:, 0:1], None, ALU.add,
                )
                nc.vector.tensor_scalar(
                    tq[:, 0, 512:], ps[:, 512:], bq[:, 0:1], None, ALU.add,
                )

            def emit_vnat_merged(t):
                vn_ready.add((t, 0))
                vn_ready.add((t, 1))
                ps = misc_ps(1024)
                for n in range(2):
                    for kc in range(2):
                        nc.tensor.matmul(
                            ps[:, n * 512:(n + 1) * 512],
                            wv[:, kc, t * 128:(t + 1) * 128],
                            x_sb[:, kc, n * 512:(n + 1) * 512],
                            start=(kc == 0), stop=(kc == 1),
                            skip_group_check=True,
                        )
                nc.vector.tensor_scalar(
                    vnat[:, t, :], ps[:], bv[:, t:t + 1], None, ALU.add,
                )

            def emit_vt_multi(jcs):
                ps = misc_ps(512 * len(jcs))
                for idx, jc in enumerate(jcs):
                    vt_ready.add(jc)
                    for kc in range(2):
                        nc.tensor.matmul(
                            ps[:, idx * 512:(idx + 1) * 512],
                            x_sb[:, kc, jc * 128:(jc + 1) * 128],
                            wv64[:, kc, :],
                            start=(kc == 0), stop=(kc == 1),
                            skip_group_check=True,
                        )
                for idx, jc in enumerate(jcs):
                    nc.vector.tensor_tensor(
                        vT[:, jc, :], ps[:, idx * 512:(idx + 1) * 512],
                        bvb[:], ALU.add,
                    )

            # pe = depthwise conv(k=3, pad 1) on v + bias, into peacc[:, t, :]
            peacc_done = [False, False]
            pe_pending = []

            def emit_peacc(t):
                assert (t, 0) in vn_ready and (t, 1) in vn_ready
                nc.vector.tensor_scalar(
                    peacc[:, t, :], vnat[:, t, :], wpe[:, t, 1:2], bpe[:, t:t + 1],
                    ALU.mult, ALU.add,
                )
                tmp_l = wpool.tile([128, 1024], BF16, tag="pel", name=f"pel{t}")
                nc.vector.tensor_scalar(
                    tmp_l[:, :1023], vnat[:, t, :1023], wpe[:, t, 0:1], None,
                    ALU.mult,
                )
                nc.vector.tensor_tensor(
                    peacc[:, t, 1:], peacc[:, t, 1:], tmp_l[:, :1023], ALU.add,
                )
                tmp_r = wpool.tile([128, 1024], BF16, tag="per", name=f"per{t}")
                nc.vector.tensor_scalar(
                    tmp_r[:, :1023], vnat[:, t, 1:], wpe[:, t, 2:3], None,
                    ALU.mult,
                )
                nc.vector.tensor_tensor(
                    peacc[:, t, :1023], peacc[:, t, :1023], tmp_r[:, :1023], ALU.add,
                )
                peacc_done[t] = True
                for (tt_, nn_) in [p for p in pe_pending if p[0] == t]:
                    pe_pending.remove((tt_, nn_))
                    emit_pe_add(tt_, nn_)

            def emit_pe_add(t, n):
                nc.vector.tensor_tensor(
                    ybf[:, t, n * 512:(n + 1) * 512],
                    ybf[:, t, n * 512:(n + 1) * 512],
                    peacc[:, t, n * 512:(n + 1) * 512], ALU.add,
                )

            def emit_proj_pair(n):
                # both mo units in one (128,1024) S-slot borrow
                ps = misc_ps(1024)
                for mo in range(2):
                    for kc in range(2):
                        nc.tensor.matmul(
                            ps[:, mo * 512:(mo + 1) * 512],
                            wpt[:, kc, mo * 128:(mo + 1) * 128],
                            ybf[:, kc, n * 512:(n + 1) * 512],
                            start=(kc == 0), stop=(kc == 1),
                            skip_group_check=True,
                        )
                for mo in range(2):
                    nc.scalar.activation(
                        zout[:, mo, n * 512:(n + 1) * 512],
                        ps[:, mo * 512:(mo + 1) * 512],
                        AF.Identity, bias=bproj[:, mo:mo + 1],
                    )
                    nc.sync.dma_start(
                        out_d.ap()[:, mo, n * 512:(n + 1) * 512],
                        zout[:, mo, n * 512:(n + 1) * 512],
                    )

            # ---- phase 2: software-pipelined attention, AV one step lagged --
            steps = [
                (t, n, jc)
                for t in range(2) for n in range(2) for jc in range(8)
            ]
            av_tiles = {}

            def emit_s_step(step):
                t, n, jc = step
                tiles = []
                assert (id(tk), t, 0) in kq_ready and (id(tq), t, n) in kq_ready
                assert jc < 4 or (id(tk), t, 1) in kq_ready
                for p in range(2):
                    s_ps = ps_s.tile(
                        [128, 1024], F32, tag="S", name=f"s_{t}_{n}_{jc}_{p}"
                    )
                    for gg in range(2):
                        g = 2 * p + gg
                        nc.tensor.matmul(
                            s_ps[:, gg * 512:(gg + 1) * 512],
                            tk[32 * g:32 * g + 16, t, jc * 128:(jc + 1) * 128],
                            tq[32 * g:32 * g + 16, t, n * 512:(n + 1) * 512],
                            start=True, stop=True,
                            tile_position=(32 * g, 0),
                        )
                    tiles.append(s_ps)
                return tiles

            def finish_tn(t, n):
                # av layout per 512-col half: heads (g_even, g_odd) at
                # partitions [0:32] / [64:96]; partitions [32:64] / [96:128]
                # hold 32 REPLICATED copies of that head's denominator (the
                # 32 ones-columns of the stationary) — the 1/d "broadcast"
                # comes out of the full-tile reciprocal for free.
                av = av_tiles.pop((t, n))
                rsb = wpool.tile([128, 1024], F32, tag="rsb", bufs=2,
                                 name=f"rsb{t}{n}")
                nc.vector.reciprocal_approx_fast(rsb[:], av[:])
                for g in range(4):
                    base = 64 * (g % 2)
                    half = (g // 2) * 512
                    nc.vector.tensor_tensor(
                        ybf[32 * g:32 * g + 32, t, n * 512:(n + 1) * 512],
                        av[base:base + 32, half:half + 512],
                        rsb[base + 32:base + 64, half:half + 512], ALU.mult,
                    )
                if peacc_done[t]:
                    emit_pe_add(t, n)
                else:
                    pe_pending.append((t, n))

            def emit_av(step, e_sb):
                # 4 AV matmuls; the 33rd stationary column (ones) accumulates
                # the softmax denominator at partitions 32 / 96 for free.
                t, n, jc = step
                assert jc in vt_ready, (t, n, jc)
                if (t, n) not in av_tiles:
                    av_tiles[(t, n)] = ps_av.tile(
                        [128, 1024], F32, tag="av", name=f"av_{t}_{n}"
                    )
                av = av_tiles[(t, n)]
                for g in range(4):
                    h = 4 * t + g
                    nc.tensor.matmul(
                        av[64 * (g % 2):64 * (g % 2) + 64,
                           (g // 2) * 512:(g // 2) * 512 + 512],
                        vT[:, jc, 64 * h:64 * h + 64],
                        e_sb[g // 2][:, (g % 2) * 512:(g % 2 + 1) * 512],
                        start=(jc == 0), stop=(jc == 7),
                        tile_position=(0, 64 * (g % 2)),
                        skip_group_check=True,
                    )
                if jc == 7:
                    finish_tn(t, n)

            # minimal front for step (0, 0, 0)
            kq_ready.add((id(tk), 0, 0))
            ps = misc_ps()
            for kc in range(2):
                nc.tensor.matmul(
                    ps[:, :128], wk[:, kc, :128], x_sb[:, kc, :128],
                    start=(kc == 0), stop=(kc == 1),
                )
            nc.vector.tensor_scalar(
                tk[:, 0, :128], ps[:, :128], bk[:, 0:1], None, ALU.add,
            )
            emit_kq(0, 0, wq, bq, tq)

            extras = {
                0: [lambda: emit_vt_multi([0, 1])],
                1: [lambda: emit_kq01_pair()],
                2: [lambda: emit_vt_multi([2, 3])],
                4: [lambda: emit_vt_multi([4, 5])],
                5: [lambda: emit_vt_multi([6, 7])],
                6: [lambda: emit_vnat_merged(0)],
                7: [lambda: emit_peacc(0)],
                9: [lambda: emit_kq_merged(1, wk, bk, tk)],
                11: [lambda: emit_kq_merged(1, wq, bq, tq)],
                13: [lambda: emit_vnat_merged(1)],
                14: [lambda: emit_peacc(1)],
            }

            s_next = emit_s_step(steps[0])
            # tk n=0 columns 128:512
            ps = misc_ps()
            for kc in range(2):
                nc.tensor.matmul(
                    ps[:, :384], wk[:, kc, :128], x_sb[:, kc, 128:512],
                    start=(kc == 0), stop=(kc == 1),
                )
            nc.vector.tensor_scalar(
                tk[:, 0, 128:512], ps[:, :384], bk[:, 0:1], None, ALU.add,
            )
            pending_av = []
            for i, step in enumerate(steps):
                t, n, jc = step
                s_cur = s_next
                e_sb = []
                for p in range(2):
                    e = epool.tile([128, 1024], BF16, tag="E", name=f"e{i}_{p}")
                    if p == 1 and i in DVE_STEPS:
                        nc.vector._custom_dve(
                            EXP_OP, out=e[:], in0=s_cur[p][:], in1=k4c,
                            s0=EXP_A, s1=EXP_B, imm2=EXP_C,
                        )
                    else:
                        nc.scalar.activation(e[:], s_cur[p][:], AF.Exp, scale=SCALE)
                    e_sb.append(e)
                if i + 1 < len(steps):
                    s_next = emit_s_step(steps[i + 1])
                for fn in extras.pop(i, []):
                    fn()
                if len(pending_av) >= 2:
                    emit_av(*pending_av.pop(0))
                pending_av.append((step, e_sb))
            assert not extras
            # ---- tail: flush lagged AV; proj n=0 overlaps the last finishes
            emit_av(*pending_av.pop(0))
            emit_proj_pair(0)
            emit_av(*pending_av.pop(0))
            emit_proj_pair(1)

    nc.compile()
    return nc


def pack_inputs(x, w_qkv, b_qkv, w_pe, b_pe, w_proj, b_proj):
    """Host-side packing of the full inputs into per-core in_maps."""
    bf16 = ml_dtypes.bfloat16
    f32 = np.float32

    # k/q packed layouts: tile t in {0,1}; partition m = 32*g + r; head h = 4t+g.
    w_kA = np.zeros((256, 256), dtype=w_qkv.dtype)
    w_qA = np.zeros((256, 256), dtype=w_qkv.dtype)
    b_kP = np.zeros((128, 2), dtype=b_qkv.dtype)
    b_qP = np.zeros((128, 2), dtype=b_qkv.dtype)
    for t in range(2):
        for m in range(128):
            g, r = m // 32, m % 32
            h = 4 * t + g
            if r < 16:
                w_kA[:, t * 128 + m] = w_qkv[64 * h + 16 + r]
                w_qA[:, t * 128 + m] = w_qkv[64 * h + r]
                b_kP[m, t] = b_qkv[64 * h + 16 + r]
                b_qP[m, t] = b_qkv[64 * h + r]

    v_rows = np.array([64 * (c // 32) + 32 + c % 32 for c in range(256)])
    w_v = w_qkv[v_rows].T  # (256 d, 256 c)
    b_v = b_qkv[v_rows]

    # 64-col-per-head layout [v(32) | zeros(32)]; bias 1.0 on the zero cols
    # makes v^T cols 32..63 exact ones columns (denominator ride-along).
    w_v64 = np.zeros((256, 512), dtype=w_qkv.dtype)
    b_v64 = np.zeros((512,), dtype=b_qkv.dtype)
    for h in range(NH):
        w_v64[:, 64 * h:64 * h + 32] = w_v[:, 32 * h:32 * h + 32]
        b_v64[64 * h:64 * h + 32] = b_v[32 * h:32 * h + 32]
        b_v64[64 * h + 32:64 * h + 64] = 1.0

    def kpart(a):  # (256, F) -> (128, 2, F)
        return np.ascontiguousarray(a.reshape(2, 128, -1).transpose(1, 0, 2))

    def chan2(a):  # (256,) -> (128, 2)
        return np.ascontiguousarray(a.reshape(2, 128).T)

    pf = np.concatenate([
        b_kP, b_qP, chan2(b_v),
        kpart(w_pe[:, 0, :]).reshape(128, 6),
        chan2(b_pe), chan2(b_proj),
        np.full((128, 1), EXP_K4, dtype=np.float64),
    ], axis=1).astype(f32)  # (128, 17)
    common = {
        "wk": kpart(w_kA).astype(bf16),
        "wq": kpart(w_qA).astype(bf16),
        "wv": kpart(w_v).astype(bf16),
        "wv64": kpart(w_v64).astype(bf16),
        "wpt": kpart(w_proj.T).astype(bf16),
        "pf": np.ascontiguousarray(pf),
        "bvrow": np.ascontiguousarray(b_v64[None, :]).astype(f32),
    }
    in_maps = []
    for b in range(B):
        m = dict(common)
        m["x"] = kpart(x[b]).astype(bf16)
        in_maps.append(m)
    return in_maps


_CACHE = {}


def kernel(x, w_qkv, b_qkv, w_pe, b_pe, w_proj, b_proj):
    x = np.asarray(x, dtype=np.float32)
    w_qkv = np.asarray(w_qkv, dtype=np.float32)
    b_qkv = np.asarray(b_qkv, dtype=np.float32)
    w_pe = np.asarray(w_pe, dtype=np.float32)
    b_pe = np.asarray(b_pe, dtype=np.float32)
    w_proj = np.asarray(w_proj, dtype=np.float32)
    b_proj = np.asarray(b_proj, dtype=np.float32)

    if "nc" not in _CACHE:
        _CACHE["nc"] = build_kernel()
    nc = _CACHE["nc"]

    in_maps = pack_inputs(x, w_qkv, b_qkv, w_pe, b_pe, w_proj, b_proj)

    trace = os.environ.get("BASS_KERNEL_TRACE", "") == "1"
    if trace:
        _install_ntff_shim()
    res = run_bass_kernel_spmd(
        nc, in_maps, core_ids=list(range(B)), trace=trace,
    )
    if trace:
        _CACHE["last_result"] = res

    out = np.empty((B, DIM, L), dtype=np.float32)
    for b in range(B):
        z = res.results[b]["out"]  # (128, 2, 1024) bf16
        out[b] = z.astype(np.float32).transpose(1, 0, 2).reshape(DIM, L)
    return out


# revision 17
# speedup vs baseline: 1.1667x; 1.1667x over previous
"""Trainium2 Bass kernel for nn_Attention_13778255085887 — stage 2.

Dense multi-head attention block (EfficientViT-style):
  qkv 1x1 conv -> per-head softmax(q^T k * scale) -> v @ attn^T
  + depthwise conv(k=3) positional encoding on v -> proj 1x1 conv.

Shapes: B=8, dim=256, L=1024, heads=8, key_dim=16, head_dim=32.

Data-parallel over B across the 8 NeuronCores (zero collectives).

Stage-2 design vs the 93us baseline:
  - exp split across ScalarE (ACTIVATE) and VectorE (custom fused DVE op
    EXP4S_ANT: monic-cubic^4 in one 1-elem/cycle pass) so neither engine
    owns the full 8.4M-element softmax.
  - the softmax denominator rides the AV matmul as a 33rd ones-column of
    the v^T stationary (d lands on partitions 32/96 of the av psum tile),
    eliminating the ones^T@E "d quad" that used to re-stream every E tile
    through the PE (-65536 PE cycles).
  - AV quads run one step LAGGED (step i emits AV for step i-1's E) so the
    PE never waits on exp and keeps its p-state high.
  - finish: one full-tile reciprocal_approx_fast + gpsimd partition
    broadcasts of the 1/d rows + per-head multiplies (gpsimd off the
    critical path, VectorE for the last group's tail).
PSUM budget: 3 S slots (6 banks) + one (128,1024) av accumulator (2).
"""

import os

import ml_dtypes
import numpy as np

import concourse.bass as bass
import concourse.mybir as mybir
import concourse.tile as tile
from concourse import bacc
from concourse.bass_utils import run_bass_kernel_spmd

BF16 = mybir.dt.bfloat16
F32 = mybir.dt.float32
AF = mybir.ActivationFunctionType
ALU = mybir.AluOpType

NH, KD, HD = 8, 16, 32
DIM, L, B = 256, 1024, 8
SCALE = KD ** -0.5  # 0.25

# ---- custom fused DVE op: E = exp(S/4) in one VectorE pass ----------------
# exp(S/4) = (k*(S^3 + A S^2 + B S + C))^4 with the monic cubic approximating
# exp(S/16)/k on S in [-7, 7] (actual |S| <= 6); rel err ~8e-4 + bf16 round.
# 8 ALU stages: add,mul,add,mul,add,sq,sq,mul(C3); C3=k^4 spills via in1.
EXP_A = 49.225494348115454
EXP_B = 1551.4792245875115
EXP_C = 24808.966983048205
EXP_K4 = 2.637892201733194e-18


def _register_exp_op():
    import concourse.dve_ops as dve_ops
    from concourse.dve_ops import DveOp
    from concourse.dve_spec import (
        C0, C1, C2, C3, Spec, Src0, _has_src1, _spill_c3_to_src1, lower, sq,
    )
    from concourse.dve_uop import DveOpSpec

    name = "EXP4S_ANT"
    if name in dve_ops._SUB_OPCODE_FOR_NAME:
        return next(op for op in dve_ops.OPS if op.name == name)
    row = dve_ops._CUSTOM_DVE_ROW_BASE + len(dve_ops.OPS)
    assert row < 0x20
    dve_ops._SUB_OPCODE_FOR_NAME[name] = row
    body = _spill_c3_to_src1(
        sq(sq(((Src0 + C0) * Src0 + C1) * Src0 + C2)) * C3
    )
    spec = Spec(
        body=body,
        reference=lambda in0, in1, s0, s1, imm2:
            (((((in0 + s0) * in0 + s1) * in0 + imm2) ** 2) ** 2) * in1,
    )
    shas = {
        ver: DveOpSpec(name=name, opcode=row, uops=lower(spec, ver=ver),
                       rd1_en=_has_src1(spec)).sha(ver)
        for ver in ("v3", "v4")
    }
    op = DveOp(name, spec, subdim=False, uops_sha=shas)
    dve_ops.OPS.append(op)
    dve_ops.CUSTOM_DVE_SPECS[name] = op.spec
    return op


EXP_OP = _register_exp_op()

# steps whose p=1 exp tile runs on VectorE (the rest stay on ScalarE, which
# keeps both tiles on the startup/finish/transition steps)
DVE_STEPS = frozenset(
    [3, 4, 5, 6, 9, 10, 11, 12, 13, 14, 17, 18, 19, 20, 21, 22,
     25, 26, 27, 28, 29]
)


def _install_ntff_shim():
    """Optionally register the axon NTFF profiling hook (for trace=True)."""
    import sys
    import types

    name = "antenv.axon_hooks"
    if name in sys.modules:
        return
    try:
        import antenv
        from trn_agent_boot.trn_boot import _ntff_profile_via_ctypes
    except ImportError:
        return
    hooks = types.ModuleType(name)
    hooks._the_hook = None
    hooks.set_axon_ntff_profile_hook = lambda h: setattr(hooks, "_the_hook", h)
    hooks.get_axon_ntff_profile_hook = lambda: hooks._the_hook
    sys.modules[name] = hooks
    antenv.axon_hooks = hooks
    so = "/opt/axon/libaxon_pjrt.so"
    if os.path.exists(so):
        hook = _ntff_profile_via_ctypes(so)
        if hook is not None:
            hooks.set_axon_ntff_profile_hook(hook)


def build_kernel() -> bass.Bass:
    nc = bacc.Bacc("TRN2", target_bir_lowering=False, debug=False, num_devices=8)

    # ---- DRAM I/O (per-core shard; weights replicated) ----
    x_d = nc.dram_tensor("x", (128, 2, 1024), BF16, kind="ExternalInput")
    wk_d = nc.dram_tensor("wk", (128, 2, 256), BF16, kind="ExternalInput")
    wq_d = nc.dram_tensor("wq", (128, 2, 256), BF16, kind="ExternalInput")
    wv_d = nc.dram_tensor("wv", (128, 2, 256), BF16, kind="ExternalInput")
    # wv64 columns per head: [v(32) | zeros(32)]; zero cols + bias 1.0 make
    # v^T cols 32..63 exact ones columns (denominator ride-along: every one of
    # them accumulates d, so the whole av tile stays finite for the recip).
    wv64_d = nc.dram_tensor("wv64", (128, 2, 512), BF16, kind="ExternalInput")
    wpt_d = nc.dram_tensor("wpt", (128, 2, 256), BF16, kind="ExternalInput")
    # [bk(2) | bq(2) | bv(2) | wpe(6) | bpe(2) | bproj(2) | k4(1)]
    pf_d = nc.dram_tensor("pf", (128, 17), F32, kind="ExternalInput")
    bvrow_d = nc.dram_tensor("bvrow", (1, 512), F32, kind="ExternalInput")
    out_d = nc.dram_tensor("out", (128, 2, 1024), BF16, kind="ExternalOutput")

    with tile.TileContext(nc) as tc:
        with (
            tc.tile_pool(name="const", bufs=1) as cpool,
            tc.tile_pool(name="work", bufs=3) as wpool,
            tc.tile_pool(name="epool", bufs=10) as epool,
            tc.tile_pool(name="ps_s", bufs=3, space="PSUM") as ps_s,
            tc.tile_pool(name="ps_av", bufs=1, space="PSUM") as ps_av,
        ):
            # phase-1/3 matmuls borrow the S pool's (128, 1024) slots.
            _misc_n = [0]

            def misc_ps(width=512):
                _misc_n[0] += 1
                return ps_s.tile(
                    [128, 1024], F32, tag="S", name=f"misc{_misc_n[0]}"
                )[:, :width]

            # ---- load constants / activations ----
            x_sb = cpool.tile([128, 2, 1024], BF16, tag="x")
            wk = cpool.tile([128, 2, 256], BF16, tag="wk")
            wq = cpool.tile([128, 2, 256], BF16, tag="wq")
            wv = cpool.tile([128, 2, 256], BF16, tag="wv")
            wv64 = cpool.tile([128, 2, 512], BF16, tag="wv64")
            wpt = cpool.tile([128, 2, 256], BF16, tag="wpt")
            pf = cpool.tile([128, 17], F32, tag="pf")
            bvrow = cpool.tile([1, 512], F32, tag="bvrow")
            bk, bq, bv = pf[:, 0:2], pf[:, 2:4], pf[:, 4:6]
            wpe = pf[:, 6:12].rearrange("p (t k) -> p t k", t=2)
            bpe, bproj = pf[:, 12:14], pf[:, 14:16]
            k4c = pf[:, 16:17]

            # critical-path loads on the sync queue; bulk on the Pool queue
            nc.sync.dma_start(wk[:, :, :128], wk_d.ap()[:, :, :128])
            nc.sync.dma_start(wq[:, :, :128], wq_d.ap()[:, :, :128])
            nc.sync.dma_start(pf[:], pf_d.ap())
            nc.sync.dma_start(x_sb[:, :, :512], x_d.ap()[:, :, :512])
            nc.gpsimd.dma_start(wv64[:], wv64_d.ap())
            nc.sync.dma_start(wk[:, :, 128:], wk_d.ap()[:, :, 128:])
            nc.sync.dma_start(wq[:, :, 128:], wq_d.ap()[:, :, 128:])
            nc.gpsimd.dma_start(x_sb[:, :, 512:], x_d.ap()[:, :, 512:])
            nc.gpsimd.dma_start(wv[:], wv_d.ap())
            nc.gpsimd.dma_start(wpt[:], wpt_d.ap())
            nc.gpsimd.dma_start(bvrow[:], bvrow_d.ap())

            # persistent intermediates
            tk = cpool.tile([128, 2, 1024], BF16, tag="tk")        # packed k
            tq = cpool.tile([128, 2, 1024], BF16, tag="tq")        # packed q
            vnat = cpool.tile([128, 2, 1024], BF16, tag="vnat")    # v, natural
            vT = cpool.tile([128, 8, 512], BF16, tag="vT")         # v^T + ones
            bvb = cpool.tile([128, 512], F32, tag="bvb")           # b_v row-bcast
            peacc = cpool.tile([128, 2, 1024], BF16, tag="peacc")  # pe conv terms
            ybf = cpool.tile([128, 2, 1024], BF16, tag="ybf")      # y = av*R + pe
            zout = cpool.tile([128, 2, 1024], BF16, tag="zout")

            nc.gpsimd.partition_broadcast(bvb[:], bvrow[:], channels=128)

            # ---- phase-1 building blocks (emitted piecemeal) ----
            kq_ready = set()
            vt_ready = set()
            vn_ready = set()

            def emit_kq(t, n, w_sb, b_sb, dst):
                kq_ready.add((id(dst), t, n))
                ps = misc_ps()
                for kc in range(2):
                    nc.tensor.matmul(
                        ps[:], w_sb[:, kc, t * 128:(t + 1) * 128],
                        x_sb[:, kc, n * 512:(n + 1) * 512],
                        start=(kc == 0), stop=(kc == 1),
                    )
                nc.vector.tensor_scalar(
                    dst[:, t, n * 512:(n + 1) * 512], ps[:],
                    b_sb[:, t:t + 1], None, ALU.add,
                )

            def emit_kq_merged(t, w_sb, b_sb, dst):
                kq_ready.add((id(dst), t, 0))
                kq_ready.add((id(dst), t, 1))
                ps = misc_ps(1024)
                for n in range(2):
                    for kc in range(2):
                        nc.tensor.matmul(
                            ps[:, n * 512:(n + 1) * 512],
                            w_sb[:, kc, t * 128:(t + 1) * 128],
                            x_sb[:, kc, n * 512:(n + 1) * 512],
                            start=(kc == 0), stop=(kc == 1),
                            skip_group_check=True,
                        )
                nc.vector.tensor_scalar(
                    dst[:, t, :], ps[:], b_sb[:, t:t + 1], None, ALU.add,
                )

            def emit_kq01_pair():
                # n=1 halves of t=0 k and q, one borrow
                kq_ready.add((id(tk), 0, 1))
                kq_ready.add((id(tq), 0, 1))
                ps = misc_ps(1024)
                for h, w_sb in enumerate([wk, wq]):
                    for kc in range(2):
                        nc.tensor.matmul(
                            ps[:, h * 512:(h + 1) * 512],
                            w_sb[:, kc, :128],
                            x_sb[:, kc, 512:],
                            start=(kc == 0), stop=(kc == 1),
                            skip_group_check=True,
                        )
                nc.vector.tensor_scalar(
                    tk[:, 0, 512:], ps[:, :512], bk[:, 0:1], None, ALU.add,
                )
                nc.vector.tensor_scalar(
                    tq[:, 0, 512:], ps[:, 512:], bq[:, 0:1], None, ALU.add,
                )

            def emit_vnat_merged(t):
                vn_ready.add((t, 0))
                vn_ready.add((t, 1))
                ps = misc_ps(1024)
                for n in range(2):
                    for kc in range(2):
                        nc.tensor.matmul(
                            ps[:, n * 512:(n + 1) * 512],
                            wv[:, kc, t * 128:(t + 1) * 128],
                            x_sb[:, kc, n * 512:(n + 1) * 512],
                            start=(kc == 0), stop=(kc == 1),
                            skip_group_check=True,
                        )
                nc.vector.tensor_scalar(
                    vnat[:, t, :], ps[:], bv[:, t:t + 1], None, ALU.add,
                )

            def emit_vt_multi(jcs):
                ps = misc_ps(512 * len(jcs))
                for idx, jc in enumerate(jcs):
                    vt_ready.add(jc)
                    for kc in range(2):
                        nc.tensor.matmul(
                            ps[:, idx * 512:(idx + 1) * 512],
                            x_sb[:, kc, jc * 128:(jc + 1) * 128],
                            wv64[:, kc, :],
                            start=(kc == 0), stop=(kc == 1),
                            skip_group_check=True,
                        )
                for idx, jc in enumerate(jcs):
                    nc.vector.tensor_tensor(
                        vT[:, jc, :], ps[:, idx * 512:(idx + 1) * 512],
                        bvb[:], ALU.add,
                    )

            # pe = depthwise conv(k=3, pad 1) on v + bias, into peacc[:, t, :]
            peacc_done = [False, False]
            pe_pending = []

            def emit_peacc(t):
                assert (t, 0) in vn_ready and (t, 1) in vn_ready
                nc.vector.tensor_scalar(
                    peacc[:, t, :], vnat[:, t, :], wpe[:, t, 1:2], bpe[:, t:t + 1],
                    ALU.mult, ALU.add,
                )
                tmp_l = wpool.tile([128, 1024], BF16, tag="pel", name=f"pel{t}")
                nc.vector.tensor_scalar(
                    tmp_l[:, :1023], vnat[:, t, :1023], wpe[:, t, 0:1], None,
                    ALU.mult,
                )
                nc.vector.tensor_tensor(
                    peacc[:, t, 1:], peacc[:, t, 1:], tmp_l[:, :1023], ALU.add,
                )
                tmp_r = wpool.tile([128, 1024], BF16, tag="per", name=f"per{t}")
                nc.vector.tensor_scalar(
                    tmp_r[:, :1023], vnat[:, t, 1:], wpe[:, t, 2:3], None,
                    ALU.mult,
                )
                nc.vector.tensor_tensor(
                    peacc[:, t, :1023], peacc[:, t, :1023], tmp_r[:, :1023], ALU.add,
                )
                peacc_done[t] = True
                for (tt_, nn_) in [p for p in pe_pending if p[0] == t]:
                    pe_pending.remove((tt_, nn_))
                    emit_pe_add(tt_, nn_)

            def emit_pe_add(t, n):
                nc.vector.tensor_tensor(
                    ybf[:, t, n * 512:(n + 1) * 512],
                    ybf[:, t, n * 512:(n + 1) * 512],
                    peacc[:, t, n * 512:(n + 1) * 512], ALU.add,
                )

            def emit_proj_pair(n):
                # both mo units in one (128,1024) S-slot borrow
                ps = misc_ps(1024)
                for mo in range(2):
                    for kc in range(2):
                        nc.tensor.matmul(
                            ps[:, mo * 512:(mo + 1) * 512],
                            wpt[:, kc, mo * 128:(mo + 1) * 128],
                            ybf[:, kc, n * 512:(n + 1) * 512],
                            start=(kc == 0), stop=(kc == 1),
                            skip_group_check=True,
                        )
                for mo in range(2):
                    nc.scalar.activation(
                        zout[:, mo, n * 512:(n + 1) * 512],
                        ps[:, mo * 512:(mo + 1) * 512],
                        AF.Identity, bias=bproj[:, mo:mo + 1],
                    )
                    nc.sync.dma_start(
                        out_d.ap()[:, mo, n * 512:(n + 1) * 512],
                        zout[:, mo, n * 512:(n + 1) * 512],
                    )

            # ---- phase 2: software-pipelined attention, AV one step lagged --
            steps = [
                (t, n, jc)
                for t in range(2) for n in range(2) for jc in range(8)
            ]
            av_tiles = {}

            def emit_s_step(step):
                t, n, jc = step
                tiles = []
                assert (id(tk), t, 0) in kq_ready and (id(tq), t, n) in kq_ready
                assert jc < 4 or (id(tk), t, 1) in kq_ready
                for p in range(2):
                    s_ps = ps_s.tile(
                        [128, 1024], F32, tag="S", name=f"s_{t}_{n}_{jc}_{p}"
                    )
                    for gg in range(2):
                        g = 2 * p + gg
                        nc.tensor.matmul(
                            s_ps[:, gg * 512:(gg + 1) * 512],
                            tk[32 * g:32 * g + 16, t, jc * 128:(jc + 1) * 128],
                            tq[32 * g:32 * g + 16, t, n * 512:(n + 1) * 512],
                            start=True, stop=True,
                            tile_position=(32 * g, 0),
                        )
                    tiles.append(s_ps)
                return tiles

            def finish_tn(t, n):
                # av layout per 512-col half: heads (g_even, g_odd) at
                # partitions [0:32] / [64:96]; partitions [32:64] / [96:128]
                # hold 32 REPLICATED copies of that head's denominator (the
                # 32 ones-columns of the stationary) — the 1/d "broadcast"
                # comes out of the full-tile reciprocal for free.
                av = av_tiles.pop((t, n))
                rsb = wpool.tile([128, 1024], F32, tag="rsb", bufs=2,
                                 name=f"rsb{t}{n}")
                nc.vector.reciprocal_approx_fast(rsb[:], av[:])
                for g in range(4):
                    base = 64 * (g % 2)
                    half = (g // 2) * 512
                    nc.vector.tensor_tensor(
                        ybf[32 * g:32 * g + 32, t, n * 512:(n + 1) * 512],
                        av[base:base + 32, half:half + 512],
                        rsb[base + 32:base + 64, half:half + 512], ALU.mult,
                    )
                if peacc_done[t]:
                    emit_pe_add(t, n)
                else:
                    pe_pending.append((t, n))

            def emit_av(step, e_sb):
                # 4 AV matmuls; the 33rd stationary column (ones) accumulates
                # the softmax denominator at partitions 32 / 96 for free.
                t, n, jc = step
                assert jc in vt_ready, (t, n, jc)
                if (t, n) not in av_tiles:
                    av_tiles[(t, n)] = ps_av.tile(
                        [128, 1024], F32, tag="av", name=f"av_{t}_{n}"
                    )
                av = av_tiles[(t, n)]
                for g in range(4):
                    h = 4 * t + g
                    nc.tensor.matmul(
                        av[64 * (g % 2):64 * (g % 2) + 64,
                           (g // 2) * 512:(g // 2) * 512 + 512],
                        vT[:, jc, 64 * h:64 * h + 64],
                        e_sb[g // 2][:, (g % 2) * 512:(g % 2 + 1) * 512],
                        start=(jc == 0), stop=(jc == 7),
                        tile_position=(0, 64 * (g % 2)),
                        skip_group_check=True,
                    )
                if jc == 7:
                    finish_tn(t, n)

            # minimal front for step (0, 0, 0)
            kq_ready.add((id(tk), 0, 0))
            ps = misc_ps()
            for kc in range(2):
                nc.tensor.matmul(
                    ps[:, :128], wk[:, kc, :128], x_sb[:, kc, :128],
                    start=(kc == 0), stop=(kc == 1),
                )
            nc.vector.tensor_scalar(
                tk[:, 0, :128], ps[:, :128], bk[:, 0:1], None, ALU.add,
            )
            emit_kq(0, 0, wq, bq, tq)

            extras = {
                0: [lambda: emit_vt_multi([0, 1])],
                1: [lambda: emit_kq01_pair()],
                2: [lambda: emit_vt_multi([2, 3])],
                4: [lambda: emit_vt_multi([4, 5])],
                5: [lambda: emit_vt_multi([6, 7])],
                6: [lambda: emit_vnat_merged(0)],
                7: [lambda: emit_peacc(0)],
                9: [lambda: emit_kq_merged(1, wk, bk, tk)],
                11: [lambda: emit_kq_merged(1, wq, bq, tq)],
                13: [lambda: emit_vnat_merged(1)],
                14: [lambda: emit_peacc(1)],
            }

            s_next = emit_s_step(steps[0])
            # tk n=0 columns 128:512
            ps = misc_ps()
            for kc in range(2):
                nc.tensor.matmul(
                    ps[:, :384], wk[:, kc, :128], x_sb[:, kc, 128:512],
                    start=(kc == 0), stop=(kc == 1),
                )
            nc.vector.tensor_scalar(
                tk[:, 0, 128:512], ps[:, :384], bk[:, 0:1], None, ALU.add,
            )
            pending_av = []
            for i, step in enumerate(steps):
                t, n, jc = step
                s_cur = s_next
                e_sb = []
                for p in range(2):
                    e = epool.tile([128, 1024], BF16, tag="E", name=f"e{i}_{p}")
                    if p == 1 and i in DVE_STEPS:
                        nc.vector._custom_dve(
                            EXP_OP, out=e[:], in0=s_cur[p][:], in1=k4c,
                            s0=EXP_A, s1=EXP_B, imm2=EXP_C,
                        )
                    else:
                        nc.scalar.activation(e[:], s_cur[p][:], AF.Exp, scale=SCALE)
                    e_sb.append(e)
                if i + 1 < len(steps):
                    s_next = emit_s_step(steps[i + 1])
                for fn in extras.pop(i, []):
                    fn()
                if len(pending_av) >= 1:
                    emit_av(*pending_av.pop(0))
                pending_av.append((step, e_sb))
            assert not extras
            # ---- tail: flush lagged AV; proj n=0 overlaps the last finish
            emit_av(*pending_av.pop(0))
            emit_proj_pair(0)
            emit_proj_pair(1)

    nc.compile()
    return nc


def pack_inputs(x, w_qkv, b_qkv, w_pe, b_pe, w_proj, b_proj):
    """Host-side packing of the full inputs into per-core in_maps."""
    bf16 = ml_dtypes.bfloat16
    f32 = np.float32

    # k/q packed layouts: tile t in {0,1}; partition m = 32*g + r; head h = 4t+g.
    w_kA = np.zeros((256, 256), dtype=w_qkv.dtype)
    w_qA = np.zeros((256, 256), dtype=w_qkv.dtype)
    b_kP = np.zeros((128, 2), dtype=b_qkv.dtype)
    b_qP = np.zeros((128, 2), dtype=b_qkv.dtype)
    for t in range(2):
        for m in range(128):
            g, r = m // 32, m % 32
            h = 4 * t + g
            if r < 16:
                w_kA[:, t * 128 + m] = w_qkv[64 * h + 16 + r]
                w_qA[:, t * 128 + m] = w_qkv[64 * h + r]
                b_kP[m, t] = b_qkv[64 * h + 16 + r]
                b_qP[m, t] = b_qkv[64 * h + r]

    v_rows = np.array([64 * (c // 32) + 32 + c % 32 for c in range(256)])
    w_v = w_qkv[v_rows].T  # (256 d, 256 c)
    b_v = b_qkv[v_rows]

    # 64-col-per-head layout [v(32) | zeros(32)]; bias 1.0 on the zero cols
    # makes v^T cols 32..63 exact ones columns (denominator ride-along).
    w_v64 = np.zeros((256, 512), dtype=w_qkv.dtype)
    b_v64 = np.zeros((512,), dtype=b_qkv.dtype)
    for h in range(NH):
        w_v64[:, 64 * h:64 * h + 32] = w_v[:, 32 * h:32 * h + 32]
        b_v64[64 * h:64 * h + 32] = b_v[32 * h:32 * h + 32]
        b_v64[64 * h + 32:64 * h + 64] = 1.0

    def kpart(a):  # (256, F) -> (128, 2, F)
        return np.ascontiguousarray(a.reshape(2, 128, -1).transpose(1, 0, 2))

    def chan2(a):  # (256,) -> (128, 2)
        return np.ascontiguousarray(a.reshape(2, 128).T)

    pf = np.concatenate([
        b_kP, b_qP, chan2(b_v),
        kpart(w_pe[:, 0, :]).reshape(128, 6),
        chan2(b_pe), chan2(b_proj),
        np.full((128, 1), EXP_K4, dtype=np.float64),
    ], axis=1).astype(f32)  # (128, 17)
    common = {
        "wk": kpart(w_kA).astype(bf16),
        "wq": kpart(w_qA).astype(bf16),
        "wv": kpart(w_v).astype(bf16),
        "wv64": kpart(w_v64).astype(bf16),
        "wpt": kpart(w_proj.T).astype(bf16),
        "pf": np.ascontiguousarray(pf),
        "bvrow": np.ascontiguousarray(b_v64[None, :]).astype(f32),
    }
    in_maps = []
    for b in range(B):
        m = dict(common)
        m["x"] = kpart(x[b]).astype(bf16)
        in_maps.append(m)
    return in_maps


_CACHE = {}


def kernel(x, w_qkv, b_qkv, w_pe, b_pe, w_proj, b_proj):
    x = np.asarray(x, dtype=np.float32)
    w_qkv = np.asarray(w_qkv, dtype=np.float32)
    b_qkv = np.asarray(b_qkv, dtype=np.float32)
    w_pe = np.asarray(w_pe, dtype=np.float32)
    b_pe = np.asarray(b_pe, dtype=np.float32)
    w_proj = np.asarray(w_proj, dtype=np.float32)
    b_proj = np.asarray(b_proj, dtype=np.float32)

    if "nc" not in _CACHE:
        _CACHE["nc"] = build_kernel()
    nc = _CACHE["nc"]

    in_maps = pack_inputs(x, w_qkv, b_qkv, w_pe, b_pe, w_proj, b_proj)

    trace = os.environ.get("BASS_KERNEL_TRACE", "") == "1"
    if trace:
        _install_ntff_shim()
    res = run_bass_kernel_spmd(
        nc, in_maps, core_ids=list(range(B)), trace=trace,
    )
    if trace:
        _CACHE["last_result"] = res

    out = np.empty((B, DIM, L), dtype=np.float32)
    for b in range(B):
        z = res.results[b]["out"]  # (128, 2, 1024) bf16
        out[b] = z.astype(np.float32).transpose(1, 0, 2).reshape(DIM, L)
    return out


# revision 19
# speedup vs baseline: 1.1681x; 1.0013x over previous
"""Trainium2 Bass kernel for nn_Attention_13778255085887 — stage 2.

Dense multi-head attention block (EfficientViT-style):
  qkv 1x1 conv -> per-head softmax(q^T k * scale) -> v @ attn^T
  + depthwise conv(k=3) positional encoding on v -> proj 1x1 conv.

Shapes: B=8, dim=256, L=1024, heads=8, key_dim=16, head_dim=32.

Data-parallel over B across the 8 NeuronCores (zero collectives).

Stage-2 design vs the 93us baseline:
  - exp split across ScalarE (ACTIVATE) and VectorE (custom fused DVE op
    EXP4S_ANT: monic-cubic^4 in one 1-elem/cycle pass) so neither engine
    owns the full 8.4M-element softmax.
  - the softmax denominator rides the AV matmul as a 33rd ones-column of
    the v^T stationary (d lands on partitions 32/96 of the av psum tile),
    eliminating the ones^T@E "d quad" that used to re-stream every E tile
    through the PE (-65536 PE cycles).
  - AV quads run one step LAGGED (step i emits AV for step i-1's E) so the
    PE never waits on exp and keeps its p-state high.
  - finish: one full-tile reciprocal_approx_fast + gpsimd partition
    broadcasts of the 1/d rows + per-head multiplies (gpsimd off the
    critical path, VectorE for the last group's tail).
PSUM budget: 3 S slots (6 banks) + one (128,1024) av accumulator (2).
"""

import os

import ml_dtypes
import numpy as np

import concourse.bass as bass
import concourse.mybir as mybir
import concourse.tile as tile
from concourse import bacc
from concourse.bass_utils import run_bass_kernel_spmd

BF16 = mybir.dt.bfloat16
F32 = mybir.dt.float32
AF = mybir.ActivationFunctionType
ALU = mybir.AluOpType

NH, KD, HD = 8, 16, 32
DIM, L, B = 256, 1024, 8
SCALE = KD ** -0.5  # 0.25

# ---- custom fused DVE op: E = exp(S/4) in one VectorE pass ----------------
# exp(S/4) = (k*(S^3 + A S^2 + B S + C))^4 with the monic cubic approximating
# exp(S/16)/k on S in [-7, 7] (actual |S| <= 6); rel err ~8e-4 + bf16 round.
# 8 ALU stages: add,mul,add,mul,add,sq,sq,mul(C3); C3=k^4 spills via in1.
EXP_A = 49.225494348115454
EXP_B = 1551.4792245875115
EXP_C = 24808.966983048205
EXP_K4 = 2.637892201733194e-18


def _register_exp_op():
    import concourse.dve_ops as dve_ops
    from concourse.dve_ops import DveOp
    from concourse.dve_spec import (
        C0, C1, C2, C3, Spec, Src0, _has_src1, _spill_c3_to_src1, lower, sq,
    )
    from concourse.dve_uop import DveOpSpec

    name = "EXP4S_ANT"
    if name in dve_ops._SUB_OPCODE_FOR_NAME:
        return next(op for op in dve_ops.OPS if op.name == name)
    row = dve_ops._CUSTOM_DVE_ROW_BASE + len(dve_ops.OPS)
    assert row < 0x20
    dve_ops._SUB_OPCODE_FOR_NAME[name] = row
    body = _spill_c3_to_src1(
        sq(sq(((Src0 + C0) * Src0 + C1) * Src0 + C2)) * C3
    )
    spec = Spec(
        body=body,
        reference=lambda in0, in1, s0, s1, imm2:
            (((((in0 + s0) * in0 + s1) * in0 + imm2) ** 2) ** 2) * in1,
    )
    shas = {
        ver: DveOpSpec(name=name, opcode=row, uops=lower(spec, ver=ver),
                       rd1_en=_has_src1(spec)).sha(ver)
        for ver in ("v3", "v4")
    }
    op = DveOp(name, spec, subdim=False, uops_sha=shas)
    dve_ops.OPS.append(op)
    dve_ops.CUSTOM_DVE_SPECS[name] = op.spec
    return op


EXP_OP = _register_exp_op()

# steps whose p=1 exp tile runs on VectorE (the rest stay on ScalarE, which
# keeps both tiles on the startup/finish/transition steps)
DVE_STEPS = frozenset(
    [3, 4, 5, 6, 9, 10, 11, 12, 14, 17, 18, 19, 20, 22,
     25, 26, 27, 28, 29]
)


def _install_ntff_shim():
    """Optionally register the axon NTFF profiling hook (for trace=True)."""
    import sys
    import types

    name = "antenv.axon_hooks"
    if name in sys.modules:
        return
    try:
        import antenv
        from trn_agent_boot.trn_boot import _ntff_profile_via_ctypes
    except ImportError:
        return
    hooks = types.ModuleType(name)
    hooks._the_hook = None
    hooks.set_axon_ntff_profile_hook = lambda h: setattr(hooks, "_the_hook", h)
    hooks.get_axon_ntff_profile_hook = lambda: hooks._the_hook
    sys.modules[name] = hooks
    antenv.axon_hooks = hooks
    so = "/opt/axon/libaxon_pjrt.so"
    if os.path.exists(so):
        hook = _ntff_profile_via_ctypes(so)
        if hook is not None:
            hooks.set_axon_ntff_profile_hook(hook)


def build_kernel() -> bass.Bass:
    nc = bacc.Bacc("TRN2", target_bir_lowering=False, debug=False, num_devices=8)

    # ---- DRAM I/O (per-core shard; weights replicated) ----
    x_d = nc.dram_tensor("x", (128, 2, 1024), BF16, kind="ExternalInput")
    wk_d = nc.dram_tensor("wk", (128, 2, 256), BF16, kind="ExternalInput")
    wq_d = nc.dram_tensor("wq", (128, 2, 256), BF16, kind="ExternalInput")
    wv_d = nc.dram_tensor("wv", (128, 2, 256), BF16, kind="ExternalInput")
    # wv64 columns per head: [v(32) | zeros(32)]; zero cols + bias 1.0 make
    # v^T cols 32..63 exact ones columns (denominator ride-along: every one of
    # them accumulates d, so the whole av tile stays finite for the recip).
    wv64_d = nc.dram_tensor("wv64", (128, 2, 512), BF16, kind="ExternalInput")
    wpt_d = nc.dram_tensor("wpt", (128, 2, 256), BF16, kind="ExternalInput")
    # [bk(2) | bq(2) | bv(2) | wpe(6) | bpe(2) | bproj(2) | k4(1)]
    pf_d = nc.dram_tensor("pf", (128, 17), F32, kind="ExternalInput")
    bvrow_d = nc.dram_tensor("bvrow", (1, 512), F32, kind="ExternalInput")
    out_d = nc.dram_tensor("out", (128, 2, 1024), BF16, kind="ExternalOutput")

    with tile.TileContext(nc) as tc:
        with (
            tc.tile_pool(name="const", bufs=1) as cpool,
            tc.tile_pool(name="work", bufs=3) as wpool,
            tc.tile_pool(name="epool", bufs=10) as epool,
            tc.tile_pool(name="ps_s", bufs=3, space="PSUM") as ps_s,
            tc.tile_pool(name="ps_av", bufs=1, space="PSUM") as ps_av,
        ):
            # phase-1/3 matmuls borrow the S pool's (128, 1024) slots.
            _misc_n = [0]

            def misc_ps(width=512):
                _misc_n[0] += 1
                return ps_s.tile(
                    [128, 1024], F32, tag="S", name=f"misc{_misc_n[0]}"
                )[:, :width]

            # ---- load constants / activations ----
            x_sb = cpool.tile([128, 2, 1024], BF16, tag="x")
            wk = cpool.tile([128, 2, 256], BF16, tag="wk")
            wq = cpool.tile([128, 2, 256], BF16, tag="wq")
            wv = cpool.tile([128, 2, 256], BF16, tag="wv")
            wv64 = cpool.tile([128, 2, 512], BF16, tag="wv64")
            wpt = cpool.tile([128, 2, 256], BF16, tag="wpt")
            pf = cpool.tile([128, 17], F32, tag="pf")
            bvrow = cpool.tile([1, 512], F32, tag="bvrow")
            bk, bq, bv = pf[:, 0:2], pf[:, 2:4], pf[:, 4:6]
            wpe = pf[:, 6:12].rearrange("p (t k) -> p t k", t=2)
            bpe, bproj = pf[:, 12:14], pf[:, 14:16]
            k4c = pf[:, 16:17]

            # critical-path loads on the sync queue; bulk on the Pool queue
            nc.sync.dma_start(wk[:, :, :128], wk_d.ap()[:, :, :128])
            nc.sync.dma_start(x_sb[:, :, :128], x_d.ap()[:, :, :128])
            nc.sync.dma_start(wq[:, :, :128], wq_d.ap()[:, :, :128])
            nc.sync.dma_start(pf[:], pf_d.ap())
            nc.sync.dma_start(x_sb[:, :, 128:512], x_d.ap()[:, :, 128:512])
            nc.gpsimd.dma_start(wv64[:], wv64_d.ap())
            nc.sync.dma_start(wk[:, :, 128:], wk_d.ap()[:, :, 128:])
            nc.sync.dma_start(wq[:, :, 128:], wq_d.ap()[:, :, 128:])
            nc.gpsimd.dma_start(x_sb[:, :, 512:], x_d.ap()[:, :, 512:])
            nc.gpsimd.dma_start(wv[:], wv_d.ap())
            nc.gpsimd.dma_start(wpt[:], wpt_d.ap())
            nc.gpsimd.dma_start(bvrow[:], bvrow_d.ap())

            # persistent intermediates
            tk = cpool.tile([128, 2, 1024], BF16, tag="tk")        # packed k
            tq = cpool.tile([128, 2, 1024], BF16, tag="tq")        # packed q
            vnat = cpool.tile([128, 2, 1024], BF16, tag="vnat")    # v, natural
            vT = cpool.tile([128, 8, 512], BF16, tag="vT")         # v^T + ones
            bvb = cpool.tile([128, 512], F32, tag="bvb")           # b_v row-bcast
            peacc = cpool.tile([128, 2, 1024], BF16, tag="peacc")  # pe conv terms
            ybf = cpool.tile([128, 2, 1024], BF16, tag="ybf")      # y = av*R + pe
            zout = cpool.tile([128, 2, 1024], BF16, tag="zout")

            nc.gpsimd.partition_broadcast(bvb[:], bvrow[:], channels=128)

            # ---- phase-1 building blocks (emitted piecemeal) ----
            kq_ready = set()
            vt_ready = set()
            vn_ready = set()

            def emit_kq(t, n, w_sb, b_sb, dst):
                kq_ready.add((id(dst), t, n))
                ps = misc_ps()
                for kc in range(2):
                    nc.tensor.matmul(
                        ps[:], w_sb[:, kc, t * 128:(t + 1) * 128],
                        x_sb[:, kc, n * 512:(n + 1) * 512],
                        start=(kc == 0), stop=(kc == 1),
                    )
                nc.vector.tensor_scalar(
                    dst[:, t, n * 512:(n + 1) * 512], ps[:],
                    b_sb[:, t:t + 1], None, ALU.add,
                )

            def emit_kq_merged(t, w_sb, b_sb, dst):
                kq_ready.add((id(dst), t, 0))
                kq_ready.add((id(dst), t, 1))
                ps = misc_ps(1024)
                for n in range(2):
                    for kc in range(2):
                        nc.tensor.matmul(
                            ps[:, n * 512:(n + 1) * 512],
                            w_sb[:, kc, t * 128:(t + 1) * 128],
                            x_sb[:, kc, n * 512:(n + 1) * 512],
                            start=(kc == 0), stop=(kc == 1),
                            skip_group_check=True,
                        )
                nc.vector.tensor_scalar(
                    dst[:, t, :], ps[:], b_sb[:, t:t + 1], None, ALU.add,
                )

            def emit_kq01_pair():
                # n=1 halves of t=0 k and q, one borrow
                kq_ready.add((id(tk), 0, 1))
                kq_ready.add((id(tq), 0, 1))
                ps = misc_ps(1024)
                for h, w_sb in enumerate([wk, wq]):
                    for kc in range(2):
                        nc.tensor.matmul(
                            ps[:, h * 512:(h + 1) * 512],
                            w_sb[:, kc, :128],
                            x_sb[:, kc, 512:],
                            start=(kc == 0), stop=(kc == 1),
                            skip_group_check=True,
                        )
                nc.vector.tensor_scalar(
                    tk[:, 0, 512:], ps[:, :512], bk[:, 0:1], None, ALU.add,
                )
                nc.vector.tensor_scalar(
                    tq[:, 0, 512:], ps[:, 512:], bq[:, 0:1], None, ALU.add,
                )

            def emit_vnat_merged(t):
                vn_ready.add((t, 0))
                vn_ready.add((t, 1))
                ps = misc_ps(1024)
                for n in range(2):
                    for kc in range(2):
                        nc.tensor.matmul(
                            ps[:, n * 512:(n + 1) * 512],
                            wv[:, kc, t * 128:(t + 1) * 128],
                            x_sb[:, kc, n * 512:(n + 1) * 512],
                            start=(kc == 0), stop=(kc == 1),
                            skip_group_check=True,
                        )
                nc.vector.tensor_scalar(
                    vnat[:, t, :], ps[:], bv[:, t:t + 1], None, ALU.add,
                )

            def emit_vt_multi(jcs):
                ps = misc_ps(512 * len(jcs))
                for idx, jc in enumerate(jcs):
                    vt_ready.add(jc)
                    for kc in range(2):
                        nc.tensor.matmul(
                            ps[:, idx * 512:(idx + 1) * 512],
                            x_sb[:, kc, jc * 128:(jc + 1) * 128],
                            wv64[:, kc, :],
                            start=(kc == 0), stop=(kc == 1),
                            skip_group_check=True,
                        )
                for idx, jc in enumerate(jcs):
                    nc.vector.tensor_tensor(
                        vT[:, jc, :], ps[:, idx * 512:(idx + 1) * 512],
                        bvb[:], ALU.add,
                    )

            # pe = depthwise conv(k=3, pad 1) on v + bias, into peacc[:, t, :]
            peacc_done = [False, False]
            pe_pending = []

            def emit_peacc(t):
                assert (t, 0) in vn_ready and (t, 1) in vn_ready
                nc.vector.tensor_scalar(
                    peacc[:, t, :], vnat[:, t, :], wpe[:, t, 1:2], bpe[:, t:t + 1],
                    ALU.mult, ALU.add,
                )
                tmp_l = wpool.tile([128, 1024], BF16, tag="pel", name=f"pel{t}")
                nc.vector.tensor_scalar(
                    tmp_l[:, :1023], vnat[:, t, :1023], wpe[:, t, 0:1], None,
                    ALU.mult,
                )
                nc.vector.tensor_tensor(
                    peacc[:, t, 1:], peacc[:, t, 1:], tmp_l[:, :1023], ALU.add,
                )
                tmp_r = wpool.tile([128, 1024], BF16, tag="per", name=f"per{t}")
                nc.vector.tensor_scalar(
                    tmp_r[:, :1023], vnat[:, t, 1:], wpe[:, t, 2:3], None,
                    ALU.mult,
                )
                nc.vector.tensor_tensor(
                    peacc[:, t, :1023], peacc[:, t, :1023], tmp_r[:, :1023], ALU.add,
                )
                peacc_done[t] = True
                for (tt_, nn_) in [p for p in pe_pending if p[0] == t]:
                    pe_pending.remove((tt_, nn_))
                    emit_pe_add(tt_, nn_)

            def emit_pe_add(t, n):
                nc.vector.tensor_tensor(
                    ybf[:, t, n * 512:(n + 1) * 512],
                    ybf[:, t, n * 512:(n + 1) * 512],
                    peacc[:, t, n * 512:(n + 1) * 512], ALU.add,
                )

            def emit_proj_pair(n):
                # both mo units in one (128,1024) S-slot borrow
                ps = misc_ps(1024)
                for mo in range(2):
                    for kc in range(2):
                        nc.tensor.matmul(
                            ps[:, mo * 512:(mo + 1) * 512],
                            wpt[:, kc, mo * 128:(mo + 1) * 128],
                            ybf[:, kc, n * 512:(n + 1) * 512],
                            start=(kc == 0), stop=(kc == 1),
                            skip_group_check=True,
                        )
                for mo in range(2):
                    nc.scalar.activation(
                        zout[:, mo, n * 512:(n + 1) * 512],
                        ps[:, mo * 512:(mo + 1) * 512],
                        AF.Identity, bias=bproj[:, mo:mo + 1],
                    )
                    nc.sync.dma_start(
                        out_d.ap()[:, mo, n * 512:(n + 1) * 512],
                        zout[:, mo, n * 512:(n + 1) * 512],
                    )

            # ---- phase 2: software-pipelined attention, AV one step lagged --
            steps = [
                (t, n, jc)
                for t in range(2) for n in range(2) for jc in range(8)
            ]
            av_tiles = {}

            def emit_s_step(step):
                t, n, jc = step
                tiles = []
                assert (id(tk), t, 0) in kq_ready and (id(tq), t, n) in kq_ready
                assert jc < 4 or (id(tk), t, 1) in kq_ready
                for p in range(2):
                    s_ps = ps_s.tile(
                        [128, 1024], F32, tag="S", name=f"s_{t}_{n}_{jc}_{p}"
                    )
                    for gg in range(2):
                        g = 2 * p + gg
                        nc.tensor.matmul(
                            s_ps[:, gg * 512:(gg + 1) * 512],
                            tk[32 * g:32 * g + 16, t, jc * 128:(jc + 1) * 128],
                            tq[32 * g:32 * g + 16, t, n * 512:(n + 1) * 512],
                            start=True, stop=True,
                            tile_position=(32 * g, 0),
                        )
                    tiles.append(s_ps)
                return tiles

            def finish_tn(t, n):
                # av layout per 512-col half: heads (g_even, g_odd) at
                # partitions [0:32] / [64:96]; partitions [32:64] / [96:128]
                # hold 32 REPLICATED copies of that head's denominator (the
                # 32 ones-columns of the stationary) — the 1/d "broadcast"
                # comes out of the full-tile reciprocal for free.
                av = av_tiles.pop((t, n))
                rsb = wpool.tile([128, 1024], F32, tag="rsb", bufs=2,
                                 name=f"rsb{t}{n}")
                nc.vector.reciprocal_approx_fast(rsb[:], av[:])
                for g in range(4):
                    base = 64 * (g % 2)
                    half = (g // 2) * 512
                    nc.vector.tensor_tensor(
                        ybf[32 * g:32 * g + 32, t, n * 512:(n + 1) * 512],
                        av[base:base + 32, half:half + 512],
                        rsb[base + 32:base + 64, half:half + 512], ALU.mult,
                    )
                if peacc_done[t]:
                    emit_pe_add(t, n)
                else:
                    pe_pending.append((t, n))

            def emit_av(step, e_sb):
                # 4 AV matmuls; the 33rd stationary column (ones) accumulates
                # the softmax denominator at partitions 32 / 96 for free.
                t, n, jc = step
                assert jc in vt_ready, (t, n, jc)
                if (t, n) not in av_tiles:
                    av_tiles[(t, n)] = ps_av.tile(
                        [128, 1024], F32, tag="av", name=f"av_{t}_{n}"
                    )
                av = av_tiles[(t, n)]
                for g in range(4):
                    h = 4 * t + g
                    nc.tensor.matmul(
                        av[64 * (g % 2):64 * (g % 2) + 64,
                           (g // 2) * 512:(g // 2) * 512 + 512],
                        vT[:, jc, 64 * h:64 * h + 64],
                        e_sb[g // 2][:, (g % 2) * 512:(g % 2 + 1) * 512],
                        start=(jc == 0), stop=(jc == 7),
                        tile_position=(0, 64 * (g % 2)),
                        skip_group_check=True,
                    )
                if jc == 7:
                    finish_tn(t, n)

            # minimal front for step (0, 0, 0)
            kq_ready.add((id(tk), 0, 0))
            ps = misc_ps()
            for kc in range(2):
                nc.tensor.matmul(
                    ps[:, :128], wk[:, kc, :128], x_sb[:, kc, :128],
                    start=(kc == 0), stop=(kc == 1),
                )
            nc.vector.tensor_scalar(
                tk[:, 0, :128], ps[:, :128], bk[:, 0:1], None, ALU.add,
            )
            emit_kq(0, 0, wq, bq, tq)

            extras = {
                0: [lambda: emit_vt_multi([0, 1])],
                1: [lambda: emit_kq01_pair()],
                2: [lambda: emit_vt_multi([2, 3])],
                4: [lambda: emit_vt_multi([4, 5])],
                5: [lambda: emit_vt_multi([6, 7])],
                6: [lambda: emit_vnat_merged(0)],
                7: [lambda: emit_peacc(0)],
                9: [lambda: emit_kq_merged(1, wk, bk, tk)],
                11: [lambda: emit_kq_merged(1, wq, bq, tq)],
                13: [lambda: emit_vnat_merged(1)],
                14: [lambda: emit_peacc(1)],
            }

            s_next = emit_s_step(steps[0])
            # tk n=0 columns 128:512
            ps = misc_ps()
            for kc in range(2):
                nc.tensor.matmul(
                    ps[:, :384], wk[:, kc, :128], x_sb[:, kc, 128:512],
                    start=(kc == 0), stop=(kc == 1),
                )
            nc.vector.tensor_scalar(
                tk[:, 0, 128:512], ps[:, :384], bk[:, 0:1], None, ALU.add,
            )
            pending_av = []
            for i, step in enumerate(steps):
                t, n, jc = step
                s_cur = s_next
                e_sb = []
                for p in range(2):
                    e = epool.tile([128, 1024], BF16, tag="E", name=f"e{i}_{p}")
                    if p == 1 and i in DVE_STEPS:
                        nc.vector._custom_dve(
                            EXP_OP, out=e[:], in0=s_cur[p][:], in1=k4c,
                            s0=EXP_A, s1=EXP_B, imm2=EXP_C,
                        )
                    else:
                        nc.scalar.activation(e[:], s_cur[p][:], AF.Exp, scale=SCALE)
                    e_sb.append(e)
                if i + 1 < len(steps):
                    s_next = emit_s_step(steps[i + 1])
                for fn in extras.pop(i, []):
                    fn()
                # flush lagged AV quads (default lag 1); a fresh group's
                # jc0 is held one extra step so the previous group's finish
                # (which frees the single av accumulator) stays off the PE
                # critical path
                while pending_av:
                    j, (jstep, jebs) = pending_av[0]
                    defer = jstep[2] == 0 and j != 0
                    if j <= i - 2 or (j == i - 1 and not defer):
                        emit_av(jstep, jebs)
                        pending_av.pop(0)
                    else:
                        break
                pending_av.append((i, (step, e_sb)))
            assert not extras
            # ---- tail: flush lagged AV; proj n=0 overlaps the last finish
            emit_proj_pair(0)
            while pending_av:
                _, (jstep, jebs) = pending_av.pop(0)
                emit_av(jstep, jebs)
            emit_proj_pair(1)

    nc.compile()
    return nc


def pack_inputs(x, w_qkv, b_qkv, w_pe, b_pe, w_proj, b_proj):
    """Host-side packing of the full inputs into per-core in_maps."""
    bf16 = ml_dtypes.bfloat16
    f32 = np.float32

    # k/q packed layouts: tile t in {0,1}; partition m = 32*g + r; head h = 4t+g.
    w_kA = np.zeros((256, 256), dtype=w_qkv.dtype)
    w_qA = np.zeros((256, 256), dtype=w_qkv.dtype)
    b_kP = np.zeros((128, 2), dtype=b_qkv.dtype)
    b_qP = np.zeros((128, 2), dtype=b_qkv.dtype)
    for t in range(2):
        for m in range(128):
            g, r = m // 32, m % 32
            h = 4 * t + g
            if r < 16:
                w_kA[:, t * 128 + m] = w_qkv[64 * h + 16 + r]
                w_qA[:, t * 128 + m] = w_qkv[64 * h + r]
                b_kP[m, t] = b_qkv[64 * h + 16 + r]
                b_qP[m, t] = b_qkv[64 * h + r]

    v_rows = np.array([64 * (c // 32) + 32 + c % 32 for c in range(256)])
    w_v = w_qkv[v_rows].T  # (256 d, 256 c)
    b_v = b_qkv[v_rows]

    # 64-col-per-head layout [v(32) | zeros(32)]; bias 1.0 on the zero cols
    # makes v^T cols 32..63 exact ones columns (denominator ride-along).
    w_v64 = np.zeros((256, 512), dtype=w_qkv.dtype)
    b_v64 = np.zeros((512,), dtype=b_qkv.dtype)
    for h in range(NH):
        w_v64[:, 64 * h:64 * h + 32] = w_v[:, 32 * h:32 * h + 32]
        b_v64[64 * h:64 * h + 32] = b_v[32 * h:32 * h + 32]
        b_v64[64 * h + 32:64 * h + 64] = 1.0

    def kpart(a):  # (256, F) -> (128, 2, F)
        return np.ascontiguousarray(a.reshape(2, 128, -1).transpose(1, 0, 2))

    def chan2(a):  # (256,) -> (128, 2)
        return np.ascontiguousarray(a.reshape(2, 128).T)

    pf = np.concatenate([
        b_kP, b_qP, chan2(b_v),
        kpart(w_pe[:, 0, :]).reshape(128, 6),
        chan2(b_pe), chan2(b_proj),
        np.full((128, 1), EXP_K4, dtype=np.float64),
    ], axis=1).astype(f32)  # (128, 17)
    common = {
        "wk": kpart(w_kA).astype(bf16),
        "wq": kpart(w_qA).astype(bf16),
        "wv": kpart(w_v).astype(bf16),
        "wv64": kpart(w_v64).astype(bf16),
        "wpt": kpart(w_proj.T).astype(bf16),
        "pf": np.ascontiguousarray(pf),
        "bvrow": np.ascontiguousarray(b_v64[None, :]).astype(f32),
    }
    in_maps = []
    for b in range(B):
        m = dict(common)
        m["x"] = kpart(x[b]).astype(bf16)
        in_maps.append(m)
    return in_maps


_CACHE = {}


def kernel(x, w_qkv, b_qkv, w_pe, b_pe, w_proj, b_proj):
    x = np.asarray(x, dtype=np.float32)
    w_qkv = np.asarray(w_qkv, dtype=np.float32)
    b_qkv = np.asarray(b_qkv, dtype=np.float32)
    w_pe = np.asarray(w_pe, dtype=np.float32)
    b_pe = np.asarray(b_pe, dtype=np.float32)
    w_proj = np.asarray(w_proj, dtype=np.float32)
    b_proj = np.asarray(b_proj, dtype=np.float32)

    if "nc" not in _CACHE:
        _CACHE["nc"] = build_kernel()
    nc = _CACHE["nc"]

    in_maps = pack_inputs(x, w_qkv, b_qkv, w_pe, b_pe, w_proj, b_proj)

    trace = os.environ.get("BASS_KERNEL_TRACE", "") == "1"
    if trace:
        _install_ntff_shim()
    res = run_bass_kernel_spmd(
        nc, in_maps, core_ids=list(range(B)), trace=trace,
    )
    if trace:
        _CACHE["last_result"] = res

    out = np.empty((B, DIM, L), dtype=np.float32)
    for b in range(B):
        z = res.results[b]["out"]  # (128, 2, 1024) bf16
        out[b] = z.astype(np.float32).transpose(1, 0, 2).reshape(DIM, L)
    return out
